# revision 1
# baseline (speedup 1.0000x reference)
"""Trainium2 Bass kernel for nn_GCNLSTMRawPluginGenderHanded.

Model: 3-layer unbatched LSTM (seq=1024, in=8500, hidden=640) -> 4 GCN layers
(dense normalized adjacency) with leaky_relu + batchnorm -> segment_sum ->
concat(gender, handed) -> 3 linear layers -> [16, 1].

Strategy (8 NeuronCores, uniform SPMD program, no divergent control flow):
  - Stage A: the big input projection xW0 = x_aug @ Wih0_aug.T is t-sharded:
    core c computes steps [128c, 128c+128), then one AllGather shares all of it.
  - Rounds: the three LSTM layer scans are pipelined across cores 0/1/2
    (software pipeline, chunk = 64 steps). Every core runs the same scan code
    on its own layer's weights (cores 3-7 get zero weights); per-round chunk
    handoff goes through an AllGather of each core's chunk output.
  - Tail: GCN + BN + segment-sum + FCN computed redundantly on every core
    from the gathered layer-2 output.

kernel(**inputs) accepts the full unsharded inputs and returns [16, 1] f32.
"""
import sys

for _p in ("/opt/trn_rl_repo",):
    if _p not in sys.path:
        sys.path.insert(0, _p)

import numpy as np
import ml_dtypes

BF16 = ml_dtypes.bfloat16

# ---------------------------------------------------------------- constants
N_NODES = 1024          # LSTM sequence length == number of graph nodes
BS = 16
LENIN = 8500
H = 640                 # hidden size
G4 = 4 * H              # 2560 gate rows
P = 128                 # partitions
NJ = H // P             # 5 hidden planes
NM = G4 // P            # 20 gate row-tiles
NCORES = 8
C = 64                  # scan chunk (steps per round)
NCH = N_NODES // C      # 16 chunks
ROUNDS = NCH + 2        # 3-deep layer pipeline -> 2 fill/drain rounds
KX = LENIN // P + 1     # 67 k-tiles of padded x (8576 = 67*128)
KXA = KX + 1            # +1 bias tile -> 68
TLOC = N_NODES // NCORES  # 128 steps of xW0 computed per core in stage A
UNROLL = 8

GCN_DIMS = [(640, 320), (320, 180), (180, 90), (90, 50)]
LEAKY_SLOPE = 0.01
BN_EPS = 1e-5


def _pad_to(x, shape):
    out = np.zeros(shape, x.dtype)
    out[tuple(slice(0, s) for s in x.shape)] = x
    return out


def _tile_lhsT(wT, nk, nm):
    """[nk*P, nm*P] -> m-major tile grid flat [(m k p), P]."""
    return np.ascontiguousarray(
        wT.reshape(nk, P, nm, P).transpose(2, 0, 1, 3)
    ).reshape(nm * nk * P, P)


# =============================================================== host prep
def prep_lstm_inputs(x_in, lstm_params):
    """lstm_params: list of 3 tuples (Wih, Whh, bih, bhh) float32."""
    xT = np.zeros((KXA * P, N_NODES), np.float32)
    xT[:LENIN] = x_in.T
    xT[KX * P] = 1.0

    Wih0, _, bih0, bhh0 = lstm_params[0]
    w0T = np.zeros((KXA * P, G4), np.float32)
    w0T[:LENIN] = Wih0.T
    w0T[KX * P] = bih0 + bhh0
    w0t_tiled = _tile_lhsT(w0T, KXA, NM)

    whT_cores, wiT_cores = [], []
    for c in range(NCORES):
        if c < 3:
            whT = np.ascontiguousarray(lstm_params[c][1].T).astype(BF16)  # [H, G4]
            whT_t = _tile_lhsT(whT, NJ, NM)
        else:
            whT_t = np.zeros((NM * NJ * P, P), BF16)
        if c in (1, 2):
            Wih, _, bih, bhh = lstm_params[c]
            wiT = np.zeros(((NJ + 1) * P, G4), np.float32)
            wiT[:H] = Wih.T
            wiT[NJ * P] = bih + bhh
            wiT_t = _tile_lhsT(wiT.astype(BF16), NJ + 1, NM)
        else:
            wiT_t = np.zeros((NM * (NJ + 1) * P, P), BF16)
        whT_cores.append(np.ascontiguousarray(whT_t))
        wiT_cores.append(np.ascontiguousarray(wiT_t))

    ones_plane = np.zeros((P, C), BF16)
    ones_plane[0] = 1.0

    rmask_cores = []
    for c in range(NCORES):
        rm = np.ones((P, ROUNDS), np.float32)
        if c < ROUNDS:
            rm[:, c] = 0.0
        rmask_cores.append(rm)

    return dict(xT=xT, w0t=w0t_tiled, whT_cores=whT_cores, wiT_cores=wiT_cores,
                ones_plane=ones_plane, rmask_cores=rmask_cores)


def prep_graph_inputs(edge_index, gcn_params, fcn_params, gender, handed):
    src = np.concatenate([np.asarray(edge_index[0]), np.arange(N_NODES)]).astype(np.int64)
    dst = np.concatenate([np.asarray(edge_index[1]), np.arange(N_NODES)]).astype(np.int64)
    deg = np.zeros(N_NODES, np.float32)
    np.add.at(deg, dst, 1.0)
    dinv = 1.0 / np.sqrt(deg)
    norm = (dinv[src] * dinv[dst]).astype(np.float32)
    A = np.zeros((N_NODES, N_NODES), np.float32)
    np.add.at(A, (dst, src), norm)
    atT = _tile_lhsT(np.ascontiguousarray(A.T), 8, 8)  # lhsT grid for A @ Z

    gws, gbs = [], []
    for li, (fi, fo) in enumerate(GCN_DIMS):
        W, b = gcn_params[li]
        kf = (fi + P - 1) // P
        fop = ((fo + P - 1) // P) * P
        gws.append(np.ascontiguousarray(_pad_to(W.astype(np.float32), (kf * P, fop))))
        gbs.append(_pad_to(b.astype(np.float32).reshape(-1, 1), (fop, 1)))

    (W1, b1), (W2, b2), (W3, b3) = fcn_params
    return dict(
        atT=atT, gws=gws, gbs=gbs,
        fw1=_pad_to(W1.T.astype(np.float32), (P, 32)),
        fw2=_pad_to(W2.T.astype(np.float32), (32, 16)),
        fw3=_pad_to(W3.T.astype(np.float32), (16, 1)),
        fb1=b1.astype(np.float32).reshape(32, 1),
        fb2=b2.astype(np.float32).reshape(16, 1),
        fb3=b3.astype(np.float32).reshape(1, 1),
        gender=np.asarray(gender, np.float32), handed=np.asarray(handed, np.float32),
    )


# ============================================================ device builders
def emit_lstm_step(nc, mybir, t, whh_sb, Yh, c_sb, xw_sb, st):
    """One LSTM cell step; t is a python int or runtime ScalarValue.

    whh_sb [P, NJ*NM, P] bf16: lhsT tile (k, m) at [:, k*NM+m, :]... (m-major: m*NJ+k)
    Yh     [P, NJ, C+1] bf16: h plane j; col t holds h_{t-1}; writes h_t at col t+1
    c_sb   [P, NJ] f32 persistent cell state
    xw_sb  [P, NM, C] f32 input projection for this chunk
    st     scratch tiles dict
    """
    AF = mybir.ActivationFunctionType
    from concourse.bass import ds
    psum_ifg, psum_o = st["psum_ifg"], st["psum_o"]
    gsb, sif, gt, tmp, tanhc, go, so = (
        st["gsb"], st["sif"], st["gt"], st["tmp"], st["tanhc"], st["go"], st["so"])

    for m in range(NM):
        dst = psum_ifg[:, m:m + 1] if m < 15 else psum_o[:, m - 15:m - 14]
        for k in range(NJ):
            nc.tensor.matmul(
                dst,
                whh_sb[:, m * NJ + k, :],
                Yh[:, k, ds(t, 1)],
                start=(k == 0), stop=(k == NJ - 1),
            )

    # epilogue: i,f,g part first (overlaps the PE 'o'-gate matmuls in HW)
    nc.vector.tensor_add(out=gsb, in0=psum_ifg, in1=xw_sb[:, 0:15, ds(t, 1)])
    nc.scalar.activation(sif, gsb[:, 0:10], AF.Sigmoid)
    nc.scalar.activation(gt, gsb[:, 10:15], AF.Tanh)
    nc.vector.tensor_mul(out=tmp, in0=sif[:, 0:5], in1=gt)       # i * g~
    nc.vector.tensor_mul(out=c_sb, in0=sif[:, 5:10], in1=c_sb)   # f * c
    nc.vector.tensor_add(out=c_sb, in0=c_sb, in1=tmp)
    nc.scalar.activation(tanhc, c_sb, AF.Tanh)
    nc.vector.tensor_add(out=go, in0=psum_o, in1=xw_sb[:, 15:20, ds(t, 1)])
    nc.scalar.activation(so, go, AF.Sigmoid)
    nc.vector.tensor_mul(out=Yh[:, 0:NJ, ds(t + 1, 1)], in0=so, in1=tanhc)


def alloc_step_scratch(pool, psum_pool, mybir):
    f32 = mybir.dt.float32
    return dict(
        psum_ifg=psum_pool.tile([P, 15], f32, tag="psum_ifg", name="psum_ifg"),
        psum_o=psum_pool.tile([P, 5], f32, tag="psum_o", name="psum_o"),
        gsb=pool.tile([P, 15], f32, tag="gsb", name="gsb"),
        sif=pool.tile([P, 10], f32, tag="sif", name="sif"),
        gt=pool.tile([P, 5], f32, tag="gt", name="gt"),
        tmp=pool.tile([P, 5], f32, tag="tmp", name="tmp"),
        tanhc=pool.tile([P, 5], f32, tag="tanhc", name="tanhc"),
        go=pool.tile([P, 5], f32, tag="go", name="go"),
        so=pool.tile([P, 5], f32, tag="so", name="so"),
    )


def emit_scan_chunk(nc, tc, mybir, whh_sb, Yh, c_sb, xw_sb, st):
    """Scan C steps with a dynamic loop (UNROLL steps per iteration)."""
    with tc.For_i(0, C, UNROLL, hint_engines=(mybir.EngineType.PE,)) as iv:
        for dt in range(UNROLL):
            emit_lstm_step(nc, mybir, iv + dt, whh_sb, Yh, c_sb, xw_sb, st)


def emit_gcn_tail(nc, tc, mybir, gio, y2_src_ap, out_ap):
    """GCN + BN + segsum + FCN. y2_src_ap: DRAM AP viewable as the layer-2
    output planes, rearranged by caller to [P, NJ, N_NODES] order.
    gio: dict of DRAM APs for graph-side inputs."""
    AF = mybir.ActivationFunctionType
    f32, bf16 = mybir.dt.float32, mybir.dt.bfloat16
    from concourse.masks import make_identity

    with tc.tile_pool(name="gcn_sbuf", bufs=1) as pool, \
         tc.tile_pool(name="gcn_w", bufs=1) as wpool, \
         tc.tile_pool(name="gcn_ps", bufs=2, space="PSUM") as pspool, \
         tc.tile_pool(name="gcn_ps2", bufs=2, space="PSUM") as pspool2:
        ident = wpool.tile([P, P], f32)
        make_identity(nc, ident)

        atT_sb = wpool.tile([P, 64, P], f32)
        nc.sync.dma_start(out=atT_sb, in_=gio["atT"].rearrange(
            "(n p) c -> p n c", n=64, p=P))

        # x^T planes, bf16 [P, kf, 1024]; y2_src_ap is [P, NJ, NCH, C]
        kf0 = NJ
        xsb = pool.tile([P, kf0, N_NODES], f32, tag="xsb0")
        for j in range(NJ):
            nc.gpsimd.dma_start(
                out=xsb[:, j, :].rearrange("p (q c) -> p q c", q=NCH, c=C),
                in_=y2_src_ap[:, j])

        for li, (fi, fo) in enumerate(GCN_DIMS):
            kf = (fi + P - 1) // P
            nfb = (fo + P - 1) // P
            fop = nfb * P
            gw_sb = wpool.tile([P, kf, fop], f32, tag=f"gw{li}")
            nc.sync.dma_start(out=gw_sb, in_=gio["gws"][li].rearrange(
                "(k p) f -> p k f", k=kf, p=P))
            gb_sb = wpool.tile([P, nfb], f32, tag=f"gb{li}")
            nc.sync.dma_start(out=gb_sb, in_=gio["gbs"][li].rearrange(
                "(b p) one -> p b one", b=nfb, p=P))

            # Z = X @ W  (node-major), then M = A @ Z (node-major)
            zsb = pool.tile([P, 8, fop], f32, tag="zsb")
            for nm in range(8):
                psz = pspool.tile([P, fop], f32, tag="psz")
                for k in range(kf):
                    nc.tensor.matmul(psz, xsb[:, k, nm * P:(nm + 1) * P],
                                     gw_sb[:, k, :], start=(k == 0), stop=(k == kf - 1))
                nc.vector.tensor_copy(out=zsb[:, nm, :], in_=psz)
            mT = pool.tile([P, nfb, N_NODES], f32, tag="mT")
            for nm in range(8):
                psm = pspool.tile([P, fop], f32, tag="psm")
                for k in range(8):
                    nc.tensor.matmul(psm, atT_sb[:, nm * 8 + k, :], zsb[:, k, :],
                                     start=(k == 0), stop=(k == 7))
                msb = pool.tile([P, fop], f32, tag="msb")
                nc.vector.tensor_copy(out=msb, in_=psm)
                for fb in range(nfb):
                    pst = pspool2.tile([P, P], f32, tag="pst")
                    nc.tensor.transpose(pst, msb[:, fb * P:(fb + 1) * P], ident)
                    nc.vector.tensor_copy(out=mT[:, fb, nm * P:(nm + 1) * P], in_=pst)

            # feat-major: bias + leaky_relu + batchnorm -> next layer planes
            last = (li == len(GCN_DIMS) - 1)
            nkf_next = nfb
            xnext = pool.tile([P, nkf_next, N_NODES], f32,
                              tag=f"xsb{li + 1}")
            for fb in range(nfb):
                lk = pool.tile([P, N_NODES], f32, tag="lk")
                nc.vector.tensor_scalar(out=lk, in0=mT[:, fb, :],
                                        scalar1=gb_sb[:, fb:fb + 1], scalar2=None,
                                        op0=mybir.AluOpType.add)
                lk2 = pool.tile([P, N_NODES], f32, tag="lk2")
                nc.vector.tensor_scalar_mul(lk2, lk, LEAKY_SLOPE)
                nc.vector.tensor_max(out=lk, in0=lk, in1=lk2)
                st6 = pool.tile([P, 12], f32, tag="st6")
                nc.vector.bn_stats(st6[:, 0:6], lk[:, 0:512])
                nc.vector.bn_stats(st6[:, 6:12], lk[:, 512:1024])
                mv = pool.tile([P, 2], f32, tag="mv")
                nc.vector.bn_aggr(mv, st6)
                veps = pool.tile([P, 1], f32, tag="veps")
                nc.vector.tensor_scalar_add(veps, mv[:, 1:2], BN_EPS)
                sd = pool.tile([P, 1], f32, tag="sd")
                nc.scalar.activation(sd, veps, AF.Sqrt)
                rs = pool.tile([P, 1], f32, tag="rs")
                nc.vector.reciprocal(rs, sd)
                nc.vector.tensor_scalar(out=xnext[:, fb, :], in0=lk,
                                        scalar1=mv[:, 0:1], scalar2=rs,
                                        op0=mybir.AluOpType.subtract,
                                        op1=mybir.AluOpType.mult)
            xsb = xnext

        # segment sum over 16 contiguous 64-node graphs -> [P, 16]
        ssb = pool.tile([P, BS], f32)
        for g in range(BS):
            nc.vector.tensor_reduce(out=ssb[:, g:g + 1], in_=xsb[:, 0, 64 * g:64 * (g + 1)],
                                    axis=mybir.AxisListType.X, op=mybir.AluOpType.add)
        # gender/handed -> rows 50, 51
        nc.sync.dma_start(out=ssb[50:51, :], in_=gio["gender"].rearrange("b one -> one b"))
        nc.sync.dma_start(out=ssb[51:52, :], in_=gio["handed"].rearrange("b one -> one b"))

        # FCN in f32
        fw1 = wpool.tile([P, 32], f32)
        fw2 = wpool.tile([32, 16], f32)
        fw3 = wpool.tile([16, 1], f32)
        fb1 = wpool.tile([32, 1], f32)
        fb2 = wpool.tile([16, 1], f32)
        fb3 = wpool.tile([1, 1], f32)
        for name, t in (("fw1", fw1), ("fw2", fw2), ("fw3", fw3),
                        ("fb1", fb1), ("fb2", fb2), ("fb3", fb3)):
            nc.sync.dma_start(out=t, in_=gio[name])
        ps1 = pspool.tile([32, BS], f32, tag="fc")
        nc.tensor.matmul(ps1, fw1, ssb, start=True, stop=True)
        x1 = pool.tile([32, BS], f32)
        nc.scalar.activation(x1, ps1, AF.Identity, bias=fb1[:, 0:1])
        ps2 = pspool.tile([16, BS], f32, tag="fc")
        nc.tensor.matmul(ps2, fw2, x1, start=True, stop=True)
        x2 = pool.tile([16, BS], f32)
        nc.scalar.activation(x2, ps2, AF.Identity, bias=fb2[:, 0:1])
        ps3 = pspool.tile([1, BS], f32, tag="fc")
        nc.tensor.matmul(ps3, fw3, x2, start=True, stop=True)
        x3 = pool.tile([1, BS], f32)
        nc.scalar.activation(x3, ps3, AF.Identity, bias=fb3[:, 0:1])
        nc.sync.dma_start(out=out_ap.rearrange("b one -> one b"), in_=x3)


# ============================================================ full program
_CACHED = {}


def build_nc(reps=1):
    import concourse.bass as bass
    import concourse.mybir as mybir
    import concourse.tile as tile
    from concourse import bacc
    from concourse.bass import ds

    f32, bf16 = mybir.dt.float32, mybir.dt.bfloat16
    nc = bacc.Bacc("TRN2", target_bir_lowering=False, debug=False,
                   num_devices=NCORES)

    # ---- I/O
    din = {}
    def inp(name, shape, dt):
        din[name] = nc.dram_tensor(name, list(shape), dt, kind="ExternalInput").ap()
        return din[name]

    xt_loc = inp("xt_loc", [KXA * P, TLOC], f32)
    w0t = inp("w0t", [NM * KXA * P, P], f32)
    whT_loc = inp("whT_loc", [NM * NJ * P, P], bf16)
    wiT_loc = inp("wiT_loc", [NM * (NJ + 1) * P, P], bf16)
    ones_pl = inp("ones_plane", [P, C], bf16)
    rmask = inp("rmask", [P, ROUNDS], f32)
    xw0scale = inp("xw0scale", [P, 1], f32)
    gio = dict(
        atT=inp("atT", [64 * P, P], f32),
        gws=[inp(f"gw{i}", list(g.shape), f32) for i, g in enumerate(_GSHAPES["gws"])],
        gbs=[inp(f"gb{i}", list(g.shape), f32) for i, g in enumerate(_GSHAPES["gbs"])],
        fw1=inp("fw1", [P, 32], f32), fw2=inp("fw2", [32, 16], f32),
        fw3=inp("fw3", [16, 1], f32), fb1=inp("fb1", [32, 1], f32),
        fb2=inp("fb2", [16, 1], f32), fb3=inp("fb3", [1, 1], f32),
        gender=inp("gender", [BS, 1], f32), handed=inp("handed", [BS, 1], f32),
    )
    out_t = nc.dram_tensor("out", [BS, 1], f32, kind="ExternalOutput").ap()

    # ---- internal DRAM
    xw0_stage = nc.dram_tensor("xw0_stage", [2 * NM * P, C], f32).ap()
    xw0_ag = nc.dram_tensor("xw0_ag", [NCH * NM * P, C], f32, addr_space="Shared").ap()
    ybounce = nc.dram_tensor("ybounce", [NJ * P, C], bf16).ap()
    yag = [nc.dram_tensor(f"yag{i}", [NCORES * NJ * P, C], bf16,
                          addr_space="Shared").ap() for i in range(2)]
    y2_dram = nc.dram_tensor("y2_dram", [NCH * NJ * P, C], bf16).ap()
    dbg = globals().get("DEBUG_TAPS", False)
    if dbg:
        dbg_xw0 = nc.dram_tensor("dbg_xw0", [NCH * NM * P, C], f32,
                                 kind="ExternalOutput").ap()
        dbg_y = [nc.dram_tensor(f"dbg_y{i}", [NCH * NJ * P, C], bf16,
                                kind="ExternalOutput").ap() for i in range(2)]

    with tile.TileContext(nc) as tc:
      pid = nc.sync.partition_id()
      rank_prev = (pid + (NCORES - 1)) % NCORES
      for _rep in range(reps):
        # ================= stage A: xW0 slice (TLOC steps) + AllGather
        with tc.tile_pool(name="sa_x", bufs=1) as xpool, \
             tc.tile_pool(name="sa_w", bufs=2) as wpool, \
             tc.tile_pool(name="sa_r", bufs=2) as rpool, \
             tc.tile_pool(name="sa_ps", bufs=2, space="PSUM") as pspool:
            xsb = xpool.tile([P, KXA, TLOC], f32)
            nc.sync.dma_start(out=xsb, in_=xt_loc.rearrange("(k p) t -> p k t", k=KXA, p=P))
            w0v = w0t.rearrange("(m k p) c -> m p k c", m=NM, k=KXA, p=P)
            stv = xw0_stage.rearrange("(b m p) c -> b m p c", b=2, m=NM, p=P)
            for m in range(NM):
                wsb = wpool.tile([P, KXA, P], f32, tag="w0")
                nc.sync.dma_start(out=wsb, in_=w0v[m])
                ps = pspool.tile([P, TLOC], f32, tag="a")
                for k in range(KXA):
                    nc.tensor.matmul(ps, wsb[:, k, :], xsb[:, k, :],
                                     start=(k == 0), stop=(k == KXA - 1))
                res = rpool.tile([P, TLOC], f32, tag="res")
                nc.vector.tensor_copy(out=res, in_=ps)
                for b in range(2):
                    nc.sync.dma_start(out=stv[b, m], in_=res[:, b * C:(b + 1) * C])
        nc.gpsimd.collective_compute(
            "AllGather", mybir.AluOpType.bypass,
            replica_groups=[list(range(NCORES))],
            ins=[xw0_stage.opt()], outs=[xw0_ag.opt()])

        # ================= rounds: pipelined scans
        with tc.tile_pool(name="sc_w", bufs=1) as cwpool, \
             tc.tile_pool(name="sc_st", bufs=1) as stpool, \
             tc.tile_pool(name="sc_ch", bufs=2) as chpool, \
             tc.tile_pool(name="sc_ps", bufs=1, space="PSUM") as scps, \
             tc.tile_pool(name="sc_psx", bufs=2, space="PSUM") as scpsx:
            whh_sb = cwpool.tile([P, NM * NJ, P], bf16)
            nc.sync.dma_start(out=whh_sb, in_=whT_loc.rearrange(
                "(n p) c -> p n c", n=NM * NJ, p=P))
            wih_sb = cwpool.tile([P, NM * (NJ + 1), P], bf16)
            nc.sync.dma_start(out=wih_sb, in_=wiT_loc.rearrange(
                "(n p) c -> p n c", n=NM * (NJ + 1), p=P))
            ones_sb = cwpool.tile([P, C], bf16)
            nc.sync.dma_start(out=ones_sb, in_=ones_pl)
            rm_sb = cwpool.tile([P, ROUNDS], f32)
            nc.sync.dma_start(out=rm_sb, in_=rmask)
            x0s_sb = cwpool.tile([P, 1], f32)
            nc.sync.dma_start(out=x0s_sb, in_=xw0scale)

            c_sb = stpool.tile([P, NJ], f32)
            hcarry = stpool.tile([P, NJ], bf16)
            nc.vector.memset(c_sb, 0.0)
            nc.vector.memset(hcarry, 0.0)
            st = alloc_step_scratch(stpool, scps, mybir)

            # zero-init both yag buffers (uninitialized DRAM may hold NaNs)
            zt = stpool.tile([P, NJ, C], bf16)
            nc.vector.memset(zt, 0.0)
            for buf in range(2):
                for r in range(NCORES):
                    nc.sync.dma_start(
                        out=yag[buf][r * NJ * P:(r + 1) * NJ * P, :].rearrange(
                            "(j p) c -> p j c", j=NJ, p=P),
                        in_=zt)

            xw0v = xw0_ag.rearrange("(n p) c -> p n c", n=NCH * NM, p=P)
            for r in range(ROUNDS):
                q = (r - pid + 2 * NCH) % NCH
                xw_sb = chpool.tile([P, NM, C], f32, tag="xw")
                nc.sync.dma_start(out=xw_sb, in_=xw0v[:, ds(q * NM, NM), :])
                yp_sb = chpool.tile([P, NJ, C], bf16, tag="yp")
                nc.sync.dma_start(
                    out=yp_sb,
                    in_=yag[(r + 1) % 2].rearrange(
                        "(n p) c -> p n c", n=NCORES * NJ, p=P)[:, ds(rank_prev * NJ, NJ), :])

                # in-layer input projection: xw += WihT_loc @ [yprev; ones]
                for m in range(NM):
                    psx = scpsx.tile([P, C], f32, tag="psx")
                    for k in range(NJ + 1):
                        rhs = yp_sb[:, k, :] if k < NJ else ones_sb
                        nc.tensor.matmul(psx, wih_sb[:, m * (NJ + 1) + k, :], rhs,
                                         start=(k == 0), stop=(k == NJ))
                    nc.vector.scalar_tensor_tensor(
                        out=xw_sb[:, m, :], in0=xw_sb[:, m, :],
                        scalar=x0s_sb[:, 0:1], in1=psx,
                        op0=mybir.AluOpType.mult, op1=mybir.AluOpType.add)

                # state reset (mask column r is 0.0 exactly on core r)
                Yh = chpool.tile([P, NJ, C + 1], bf16, tag="Yh")
                nc.vector.tensor_scalar(out=Yh[:, :, 0:1], in0=hcarry,
                                        scalar1=rm_sb[:, r:r + 1], scalar2=None,
                                        op0=mybir.AluOpType.mult)
                nc.vector.tensor_scalar(out=c_sb, in0=c_sb,
                                        scalar1=rm_sb[:, r:r + 1], scalar2=None,
                                        op0=mybir.AluOpType.mult)

                if not globals().get("SKIP_SCAN", False):
                    emit_scan_chunk(nc, tc, mybir, whh_sb, Yh, c_sb, xw_sb, st)

                nc.vector.tensor_copy(out=hcarry, in_=Yh[:, :, C:C + 1])
                nc.sync.dma_start(
                    out=ybounce.rearrange("(j p) c -> p j c", j=NJ, p=P),
                    in_=Yh[:, :, 1:C + 1])
                if not globals().get("SKIP_AG", False):
                    nc.gpsimd.collective_compute(
                        "AllGather", mybir.AluOpType.bypass,
                        replica_groups=[list(range(NCORES))],
                        ins=[ybounce.opt()], outs=[yag[r % 2].opt()])
                if 2 <= r:
                    q2 = r - 2
                    nc.sync.dma_start(
                        out=y2_dram[q2 * NJ * P:(q2 + 1) * NJ * P, :],
                        in_=yag[r % 2][2 * NJ * P:3 * NJ * P, :])
                if dbg:
                    for rk in range(2):
                        if rk <= r <= rk + NCH - 1:
                            qd = r - rk
                            nc.sync.dma_start(
                                out=dbg_y[rk][qd * NJ * P:(qd + 1) * NJ * P, :],
                                in_=yag[r % 2][rk * NJ * P:(rk + 1) * NJ * P, :])
            if dbg:
                nc.sync.dma_start(out=dbg_xw0, in_=xw0_ag)

        # ================= GCN tail
        y2v = y2_dram.rearrange("(q j p) c -> p j q c", q=NCH, j=NJ, p=P)
        emit_gcn_tail(nc, tc, mybir, gio, y2v, out_t)

    nc.compile()
    return nc


_GSHAPES = dict(
    gws=[np.zeros((((fi + P - 1) // P) * P, ((fo + P - 1) // P) * P), np.float32)
         for (fi, fo) in GCN_DIMS],
    gbs=[np.zeros((((fo + P - 1) // P) * P, 1), np.float32) for (_, fo) in GCN_DIMS],
)


# ================================================================= entry
def prepare(**inputs):
    """Host prep + program build; returns (nc, in_maps)."""
    x_in = np.asarray(inputs["x_in"], np.float32)
    lstm_params = [
        (np.asarray(inputs[f"lstm_Wih{l}"], np.float32),
         np.asarray(inputs[f"lstm_Whh{l}"], np.float32),
         np.asarray(inputs[f"lstm_bih{l}"], np.float32),
         np.asarray(inputs[f"lstm_bhh{l}"], np.float32))
        for l in range(3)]
    gcn_params = [(np.asarray(inputs[f"gcn{i}_W"], np.float32),
                   np.asarray(inputs[f"gcn{i}_b"], np.float32)) for i in range(1, 5)]
    fcn_params = [(np.asarray(inputs[f"fcn{i}_W"], np.float32),
                   np.asarray(inputs[f"fcn{i}_b"], np.float32)) for i in range(1, 4)]

    lp = prep_lstm_inputs(x_in, lstm_params)
    gp = prep_graph_inputs(np.asarray(inputs["edge_index"]), gcn_params,
                           fcn_params, inputs["gender"], inputs["handed"])

    if "nc" not in _CACHED:
        _CACHED["nc"] = build_nc()
    nc = _CACHED["nc"]

    in_maps = []
    for c in range(NCORES):
        m = dict(
            xt_loc=np.ascontiguousarray(lp["xT"][:, c * TLOC:(c + 1) * TLOC]),
            w0t=lp["w0t"], whT_loc=lp["whT_cores"][c], wiT_loc=lp["wiT_cores"][c],
            ones_plane=lp["ones_plane"], rmask=lp["rmask_cores"][c],
            xw0scale=np.full((P, 1), 1.0 if c == 0 else 0.0, np.float32),
            atT=gp["atT"],
            fw1=gp["fw1"], fw2=gp["fw2"], fw3=gp["fw3"],
            fb1=gp["fb1"], fb2=gp["fb2"], fb3=gp["fb3"],
            gender=gp["gender"], handed=gp["handed"],
        )
        for i in range(4):
            m[f"gw{i}"] = gp["gws"][i]
            m[f"gb{i}"] = gp["gbs"][i]
        in_maps.append(m)
    return nc, in_maps


def kernel(**inputs):
    from concourse.bass_utils import run_bass_kernel_spmd
    import time

    nc, in_maps = prepare(**inputs)
    t0 = time.time()
    res = run_bass_kernel_spmd(nc, in_maps, list(range(NCORES)))
    _CACHED["spmd_wall_s"] = time.time() - t0
    _CACHED["exec_time_ns"] = res.exec_time_ns
    _CACHED["last_res"] = res
    return np.asarray(res.results[0]["out"], np.float32)

